# revision 17
# baseline (speedup 1.0000x reference)
"""Bahdanau additive attention on TRN2, data-parallel over batch on 8 NeuronCores.

Reference computation (per batch b):
    pre[s, :]  = W1 @ hs[s, b, :] + b1 + W2 @ hidden[b, :] + b2      # (S, H)
    energy[s]  = v . tanh(pre[s, :])                                  # (S,)
    energy     = where(mask[s, b], energy, -1e10)
    attn       = softmax(energy over s)
    ctx[b, :]  = sum_s attn[s] * hs[s, b, :]                          # (H,)

Sparsity: masked positions get attn == 0 exactly (the reference's -1e10
fill underflows to zero weight), so the host packs only the unmasked
rows of hs per batch (~50% of S) into a fixed SP-wide layout, padded
with zero columns that the device masks out of the softmax.

Per-core layout strategy (batch-sharded, 4 batches per core):
  - hst shard (BL, H, SP) fp16: h-major so the big matmul streams
    [h_in=128p, s] tiles; pre comes out as [h_out=128p, s] in PSUM.
    fp16 runs at the same PE rate as f32r but halves DMA/SBUF, and its
    10-bit mantissa keeps the energy error well under the bf16 level.
  - The v-dot runs OFF the PE: DVE accumulates acc += v_m * tanh_m
    (per-partition scalar multiply-add), then one ones-vector matmul
    per sigma block reduces acc over partitions into energy [1, SB].
  - The energy row is masked straight out of PSUM into em (per block),
    so the softmax chain is just max + exp + reciprocal.
  - hsn shard (SP, BL, H) fp16: s-major for the context matmul.
  - w1r is W1^T relaid m-major (output-chunk-major) so the first output
    chunk's weights land first and pass 1 starts early. Batch 0 copies
    its pre blocks to SBUF so nothing on the PE critical path waits for
    the q bias (computed from W2 after pass 1 starts).
"""

import os
import sys
from contextlib import ExitStack

import numpy as np
import ml_dtypes

# Fallback path for concourse; the axon sitecustomize normally provides it.
if "/opt/trn_rl_repo" not in sys.path:
    sys.path.append("/opt/trn_rl_repo")

import concourse.bass as bass
import concourse.bacc as bacc
import concourse.mybir as mybir
import concourse.tile as tile
from concourse import bass_utils

S, B, H = 2048, 32, 1024
NCORES = 8
BL = B // NCORES  # local batches per core
HK = H // 128     # 128-partition chunks of H
SB = 384          # sigma-block width
SP_DEFAULT = 3 * SB  # packed sequence length (max unmasked count ~1024+3sigma)

F32 = mybir.dt.float32
F32R = mybir.dt.float32r
U8 = mybir.dt.uint8
BF16 = mybir.dt.bfloat16
FP16 = mybir.dt.float16
AF = mybir.ActivationFunctionType
AX = mybir.AxisListType
OP = mybir.AluOpType

_CACHE = {}


def _emit(tc, aps, SP):
    nc = tc.nc
    ctx = aps["ctx_stack"]
    NSIG = SP // SB
    NT = SP // 128  # 128-row s-tiles for the context matmul
    hst, hsn, w1r, w2t, bvt, hidr, masku, ctx_out = (
        aps["hst"], aps["hsn"], aps["w1r"], aps["w2t"],
        aps["bvt"], aps["hidr"], aps["masku"], aps["ctx"],
    )

    def pool(name, bufs, space="SBUF"):
        return ctx.enter_context(tc.tile_pool(name=name, bufs=bufs, space=space))

    p_hst = pool("hst", 3)
    p_hstb = pool("hstb", 2)
    p_w1 = pool("w1", 1)
    p_w2c = pool("w2c", 8)
    p_small = pool("small", 1)
    p_pre0 = pool("pre0", 3)
    p_hsn = pool("hsn", NT + 3)
    p_tanh = pool("tanh", 3)
    p_acc = pool("acc", 4)
    p_em = pool("em", 2)
    p_mask = pool("mask", 1)
    p_ctxs = pool("ctxs", 1)
    p_attnT = pool("attnT", 2)
    p_sc = pool("sc", 2)
    p_mx = pool("mx", 2)

    pp_pre = pool("ppre", 3, space="PSUM")
    pp_en = pool("pen", 2, space="PSUM")
    pp_tr = pool("ptr", 1, space="PSUM")
    pp_ctx = pool("pctx", 2, space="PSUM")

    # ---------------- setup DMAs, spread across idle engine queues ----------
    ident = p_small.tile([1, 1], F32, tag="ident")
    nc.gpsimd.memset(ident[:], 1.0)

    # W1 (m-major relaid) and the first hst block lead the DMA queues: m0
    # chunk first, then the full first block (triggers split across
    # sync/gpsimd so the per-trigger descriptor generation doesn't
    # serialize the head), then the remaining m chunks.
    w1_sb = p_w1.tile([128, HK * H], FP16, tag="w1")
    hst_first = p_hst.tile([128, HK * SB], FP16, tag="hst", name="hst_first")
    nc.sync.dma_start(w1_sb[:, 0:H], w1r[:, 0:H])
    for k in range(HK):
        eng = nc.sync if k % 2 == 0 else nc.gpsimd
        eng.dma_start(hst_first[:, SB * k:SB * (k + 1)], hst[0, 128 * k:128 * (k + 1), 0:SB])
    for m in range(1, HK):
        nc.sync.dma_start(w1_sb[:, H * m:H * (m + 1)], w1r[:, H * m:H * (m + 1)])

    # bvt packs [b1r | b2r | vt | eye4 | ones] as (128, 3*HK+5).
    bvt_sb = p_small.tile([128, 3 * HK + 5], F32R, tag="bvt")
    nc.scalar.dma_start(bvt_sb[:], bvt[:])
    b1_sb = bvt_sb[:, 0:HK].bitcast(F32)
    b2_sb = bvt_sb[:, HK:2 * HK].bitcast(F32)
    vt_sb = bvt_sb[:, 2 * HK:3 * HK].bitcast(F32)
    eye4 = bvt_sb[0:4, 3 * HK:3 * HK + 4].bitcast(F32)
    ones_r = bvt_sb[:, 3 * HK + 4:3 * HK + 5]  # f32r column of 1.0
    # hidden (pre-swizzled on host to [p, 4k+b]), fp16 to match the W2 rows
    hid_sb = p_small.tile([128, BL * HK], FP16, tag="hidr")
    nc.scalar.dma_start(hid_sb[:], hidr[:])

    # all four batch pad-masks in one row
    mask_all = p_mask.tile([1, BL * SP], U8, tag="mask")
    nc.scalar.dma_start(mask_all[:], masku[:])

    # W2 rows on the scalar queue, gated behind w1-m0 so their descriptors
    # don't crowd the critical first-block stream; q only needs them later.
    gate_s = p_small.tile([1, 1], F32, tag="gates")
    nc.scalar.activation(gate_s[:], w1_sb[0:1, 0:1], AF.Copy)
    w2rs = []
    for k in range(HK):
        w2r = p_w2c.tile([128, H], FP16, tag="w2c", name=f"w2r{k}")
        nc.scalar.dma_start(w2r[:], w2t[128 * k:128 * (k + 1), :])
        w2rs.append(w2r)

    # Gate the gpsimd descriptor stream (hsn + batch hst) behind the last
    # w1 chunk so the head belongs to w1/hst00/w2.
    gate_g = p_small.tile([1, 1], F32, tag="gateg")
    nc.gpsimd.tensor_copy(gate_g[:], w1_sb[0:1, HK * H - 1:HK * H])

    qt_sb = p_small.tile([128, BL * HK], F32, tag="qt")

    em_t = {}
    mx_t = {}
    attnT_t = {}
    rz_t = {}
    pre0_t = {}

    # ------------- pass 1 mains: pre = W1 @ hs for one (batch, sigma-block) --
    pending = [None]  # block whose ones-reduce is deferred into the next block

    def flush_pending():
        if pending[0] is not None:
            pb, pc = pending[0]
            pending[0] = None
            _block_reduce(pb, pc)

    hstb_t = {}

    def p1_mains(b, c, to_sbuf=False, first_tile=None):
        if c == 0:
            em_t[b] = p_em.tile([1, SP], F32, tag="em", name=f"em{b}")
        if first_tile is not None:
            rhs = lambda k: first_tile[:, SB * k:SB * (k + 1)]
        elif b == 0:
            hst_c = p_hst.tile([128, HK * SB], FP16, tag="hst", name=f"hst_{b}_{c}")
            for k in range(HK):
                nc.sync.dma_start(
                    hst_c[:, SB * k:SB * (k + 1)],
                    hst[b, 128 * k:128 * (k + 1), SB * c:SB * (c + 1)],
                )
            rhs = lambda k: hst_c[:, SB * k:SB * (k + 1)]
        else:
            # batches 1..3 stream whole k-rows: 2.3KB DMA descriptors (vs
            # 768B per-block) and 8 triggers per batch instead of 24
            if c == 0:
                hstb = p_hstb.tile([128, HK * SP], FP16, tag="hstb", name=f"hstb_{b}")
                hstb_t[b] = hstb
                for k in range(HK):
                    nc.gpsimd.dma_start(
                        hstb[:, SP * k:SP * (k + 1)], hst[b, 128 * k:128 * (k + 1), :]
                    )
            hstb = hstb_t[b]
            rhs = lambda k: hstb[:, SP * k + SB * c:SP * k + SB * (c + 1)]
        if to_sbuf:
            pre0 = p_pre0.tile([128, HK * SB], F32, tag="pre0", name=f"pre0_{c}")
            pre0_t[(b, c)] = pre0
        for m in range(HK):
            if m == 1:
                # previous block's ones-reduce lands here so its tanh/STT
                # chain has a full m-round to drain before the PE needs it
                flush_pending()
            ppre = pp_pre.tile([128, SB], F32, tag="ppre", name=f"ppre_{b}_{c}_{m}")
            for k in range(HK):
                nc.tensor.matmul(
                    ppre[:],
                    lhsT=w1_sb[:, H * m + 128 * k:H * m + 128 * k + 128],
                    rhs=rhs(k),
                    start=(k == 0), stop=(k == HK - 1),
                )
            if to_sbuf:
                # free the PSUM buf immediately; acts run later once q lands
                nc.vector.tensor_copy(pre0[:, SB * m:SB * (m + 1)], ppre[:])
            else:
                _act_chain_m(b, c, m, ppre[:])
        if not to_sbuf:
            pending[0] = (b, c)

    acc_t = {}

    def _act_chain_m(b, c, m, src):
        """tanh(+q bias) -> th; acc += v_m * th on DVE."""
        th = p_tanh.tile([128, SB], F32, tag="tanh", name=f"th_{b}_{c}_{m}")
        nc.scalar.activation(
            th[:], src, AF.Tanh,
            bias=qt_sb[:, BL * m + b:BL * m + b + 1], scale=1.0,
        )
        if m == 0:
            acc = p_acc.tile([128, SB], F32, tag="acc", name=f"acc_{b}_{c}")
            acc_t[(b, c)] = acc
            nc.vector.tensor_scalar_mul(acc[:], th[:], vt_sb[:, 0:1])
        elif m < HK - 1:
            acc = acc_t[(b, c)]
            nc.vector.scalar_tensor_tensor(
                acc[:], th[:], vt_sb[:, m:m + 1], acc[:], op0=OP.mult, op1=OP.add,
            )
        else:
            # last step writes an f32r-rounded tile: the BIR verifier requires
            # f32r matmul operands to be explicitly rounded by their producer
            acc = acc_t.pop((b, c))
            acc_r = p_acc.tile([128, SB], F32R, tag="acc", name=f"accr_{b}_{c}")
            acc_t[(b, c)] = acc_r
            nc.vector.scalar_tensor_tensor(
                acc_r[:], th[:], vt_sb[:, m:m + 1], acc[:], op0=OP.mult, op1=OP.add,
            )

    def _block_reduce(b, c):
        """energy[1,SB] = ones^T @ acc on PE; mask it straight into em."""
        acc_r = acc_t.pop((b, c))
        pen = pp_en.tile([1, SB], F32, tag="pen", name=f"pen_{b}_{c}")
        nc.tensor.matmul(
            pen[:], lhsT=ones_r, rhs=acc_r[:], start=True, stop=True,
        )
        em = em_t[b]
        nc.vector.scalar_tensor_tensor(
            em[:, SB * c:SB * (c + 1)],
            mask_all[:, b * SP + SB * c:b * SP + SB * (c + 1)], -1e10, pen[:],
            op0=OP.mult, op1=OP.add,
        )
        # per-block running max keeps the softmax-tail reduce off [1, SP]
        if c == 0:
            mx_t[b] = p_mx.tile([1, NSIG], F32, tag="mx", name=f"mx{b}")
        nc.vector.reduce_max(mx_t[b][:, c:c + 1], em[:, SB * c:SB * (c + 1)], axis=AX.X)

    def p1_acts(b, c):
        """Deferred activation chain for the SBUF-buffered blocks."""
        pre0 = pre0_t.pop((b, c))
        for m in range(HK):
            _act_chain_m(b, c, m, pre0[:, SB * m:SB * (m + 1)])
        _block_reduce(b, c)

    # ------------- q phase: qT[h_out, b] = W2 @ hidden + b1 + b2 ------------
    def q_phase():
        qn_sb = p_small.tile([BL, H], F32, tag="qnat")
        for n in range(2):
            pq = pp_ctx.tile([BL, 512], F32, tag="pctx", name=f"pq{n}")
            for k in range(HK):
                nc.tensor.matmul(
                    pq[:],
                    lhsT=hid_sb[:, BL * k:BL * (k + 1)],
                    rhs=w2rs[k][:, 512 * n:512 * (n + 1)],
                    start=(k == 0), stop=(k == HK - 1),
                )
            nc.vector.tensor_copy(qn_sb[:, 512 * n:512 * (n + 1)], pq[:])
        ptrq = pp_tr.tile([128, BL * HK], F32, tag="ptr", name="ptrq")
        for m in range(HK):
            nc.tensor.transpose(
                ptrq[:, BL * m:BL * (m + 1)], qn_sb[:, 128 * m:128 * (m + 1)], eye4
            )
        for m in range(HK):
            nc.vector.tensor_scalar_add(
                qt_sb[:, BL * m:BL * (m + 1)], ptrq[:, BL * m:BL * (m + 1)], b1_sb[:, m:m + 1]
            )
            nc.vector.tensor_scalar_add(
                qt_sb[:, BL * m:BL * (m + 1)], qt_sb[:, BL * m:BL * (m + 1)], b2_sb[:, m:m + 1]
            )

    # ------------- masked softmax over the assembled em row ------------------
    def sm_pre(b):
        """DVE/ACT part: max, exp, Z, 1/Z. No PE work."""
        em = em_t[b]
        mx = mx_t.pop(b)
        negmax = p_sc.tile([1, 1], F32, tag="negmax", name=f"negmax{b}")
        nc.vector.reduce_max(negmax[:], mx[:], axis=AX.X, negate=True)
        zs = p_sc.tile([1, 1], F32, tag="zs", name=f"zs{b}")
        nc.scalar.activation(em[:], em[:], AF.Exp, bias=negmax[:], scale=1.0, accum_out=zs[:])
        rz = p_sc.tile([1, 1], F32, tag="rz", name=f"rz{b}")
        nc.vector.reciprocal(rz[:], zs[:])
        rz_t[b] = rz

    def sm_tr(b):
        """PE part: NT tiny transposes of attn into [s-partition, 1] layout."""
        em = em_t.pop(b)
        ptr = pp_tr.tile([128, NT], F32, tag="ptr", name=f"ptr{b}")
        for cc in range(NT):
            nc.tensor.transpose(ptr[:, cc:cc + 1], em[:, 128 * cc:128 * (cc + 1)], ident[:])
        att = p_attnT.tile([128, NT], FP16, tag="attnT", name=f"attnT{b}")
        nc.vector.tensor_copy(att[:], ptr[:])
        attnT_t[b] = att

    # ------------- pass 2: context for one batch -------------
    hsn_tiles = {}

    def p2_load(b):
        tiles = []
        for t in range(NT):
            hsn_c = p_hsn.tile([128, H], FP16, tag="hsn", name=f"hsn_{b}_{t}")
            nc.gpsimd.dma_start(hsn_c[:], hsn[128 * t:128 * (t + 1), b, :])
            tiles.append(hsn_c)
        hsn_tiles[b] = tiles

    def p2_mm(b, tail=False):
        att = attnT_t.pop(b)
        rz = rz_t.pop(b)
        tiles = hsn_tiles.pop(b)
        cs = p_ctxs.tile([1, H], F32, tag="ctxs", name=f"cs{b}")
        if tail:
            # n-outer so half 0 scales+stores (on the idle scalar engine)
            # while half 1 is still accumulating on the PE
            for n in range(2):
                pc = pp_ctx.tile([1, 512], F32, tag="pctx", name=f"pctx_{b}_{n}")
                for t, hsn_c in enumerate(tiles):
                    nc.tensor.matmul(
                        pc[:],
                        lhsT=att[:, t:t + 1],
                        rhs=hsn_c[:, 512 * n:512 * (n + 1)],
                        start=(t == 0), stop=(t == NT - 1),
                    )
                nc.scalar.activation(cs[:, 512 * n:512 * (n + 1)], pc[:], AF.Copy, scale=rz[:])
                nc.sync.dma_start(ctx_out[b:b + 1, 512 * n:512 * (n + 1)], cs[:, 512 * n:512 * (n + 1)])
            return
        # t-outer: both halves share att[:, t] as the stationary per step
        pc = [
            pp_ctx.tile([1, 512], F32, tag="pctx", name=f"pctx_{b}_{n}")
            for n in range(2)
        ]
        for t, hsn_c in enumerate(tiles):
            for n in range(2):
                nc.tensor.matmul(
                    pc[n][:],
                    lhsT=att[:, t:t + 1],
                    rhs=hsn_c[:, 512 * n:512 * (n + 1)],
                    start=(t == 0), stop=(t == NT - 1),
                )
        for n in range(2):
            nc.scalar.activation(cs[:, 512 * n:512 * (n + 1)], pc[n][:], AF.Copy, scale=rz[:])
        nc.sync.dma_start(ctx_out[b:b + 1, :], cs[:])

    # ------------- schedule -------------
    # Batch 0's pre blocks buffer through SBUF so no PE work waits on q; q
    # slots into the PE stream right after block (0,0) (W2 lands by then),
    # and batch 0's activation chains drain under the later matmul stream.
    p1_mains(0, 0, to_sbuf=True, first_tile=hst_first)
    q_phase()
    for c in range(1, NSIG):
        p1_mains(0, c, to_sbuf=True)
    for c in range(NSIG):
        p1_acts(0, c)
    p2_load(0)
    for b in range(1, BL):
        p1_mains(b, 0)
        sm_pre(b - 1)
        p1_mains(b, 1)
        sm_tr(b - 1)
        for c in range(2, NSIG):
            p1_mains(b, c)
        p2_mm(b - 1)
        p2_load(b)
    flush_pending()
    sm_pre(BL - 1)
    sm_tr(BL - 1)
    p2_mm(BL - 1, tail=True)


def build_program(SP=SP_DEFAULT):
    key = ("nc", SP)
    if key in _CACHE:
        return _CACHE[key]
    nc = bacc.Bacc("TRN2", target_bir_lowering=False, debug=False, enable_asserts=False)
    aps = {
        "hst": nc.dram_tensor("hst", (BL, H, SP), FP16, kind="ExternalInput").ap(),
        "hsn": nc.dram_tensor("hsn", (SP, BL, H), FP16, kind="ExternalInput").ap(),
        "w1r": nc.dram_tensor("w1r", (128, HK * H), FP16, kind="ExternalInput").ap(),
        "w2t": nc.dram_tensor("w2t", (H, H), FP16, kind="ExternalInput").ap(),
        "bvt": nc.dram_tensor("bvt", (128, 3 * HK + 5), F32R, kind="ExternalInput").ap(),
        "hidr": nc.dram_tensor("hidr", (128, BL * HK), FP16, kind="ExternalInput").ap(),
        "masku": nc.dram_tensor("masku", (1, BL * SP), U8, kind="ExternalInput").ap(),
        "ctx": nc.dram_tensor("ctx", (BL, H), F32, kind="ExternalOutput").ap(),
    }
    with tile.TileContext(nc) as tc:
        with ExitStack() as stack:
            aps["ctx_stack"] = stack
            _emit(tc, aps, SP)
    nc.compile()
    _CACHE[key] = nc
    return nc


def pick_sp(masks):
    maxn = int(np.asarray(masks).sum(axis=0).max())
    sp = SP_DEFAULT
    while sp < maxn:
        sp += SB
    return sp


def prep_in_maps(inputs, SP=None):
    hidden = np.ascontiguousarray(np.asarray(inputs["hidden"], dtype=np.float32))
    hs = np.asarray(inputs["hidden_sequence"], dtype=np.float32)
    masks = np.asarray(inputs["input_masks"]).astype(bool)
    if SP is None:
        SP = pick_sp(masks)
    w1t = np.asarray(inputs["W1"], dtype=np.float32).T  # (hin, hout)
    # m-major relayout: w1r[:, 1024*m + 128*k : +128] = W1T[128k:128(k+1), 128m:128(m+1)]
    w1r = np.ascontiguousarray(
        w1t.reshape(HK, 128, HK, 128).transpose(1, 2, 0, 3).reshape(128, HK * H)
        .astype(np.float16)
    )
    w2t = np.ascontiguousarray(np.asarray(inputs["W2"], dtype=np.float32).T.astype(np.float16))
    b1 = np.asarray(inputs["b1"], dtype=np.float32)
    b2 = np.asarray(inputs["b2"], dtype=np.float32)
    v = np.asarray(inputs["v"], dtype=np.float32)
    # [b1r | b2r | vt | eye4 | ones] packed as (128, 29)
    ey = np.zeros((128, 4), dtype=np.float32)
    ey[0:4, 0:4] = np.eye(4, dtype=np.float32)
    ones = np.ones((128, 1), dtype=np.float32)
    bvt_base = np.concatenate(
        [b1.reshape(HK, 128).T, b2.reshape(HK, 128).T, v.reshape(HK, 128).T, ey, ones],
        axis=1,
    )
    in_maps = []
    for ci in range(NCORES):
        g = slice(BL * ci, BL * (ci + 1))
        hg = hidden[0, g, :]  # (BL, H)
        # hidr[p, BL*k + b] = hidden[b, 128k + p]
        hidr = np.ascontiguousarray(
            hg.T.reshape(HK, 128, BL).transpose(1, 0, 2).reshape(128, HK * BL).astype(np.float16)
        )
        hst_p = np.zeros((BL, H, SP), dtype=np.float16)
        hsn_p = np.zeros((SP, BL, H), dtype=np.float16)
        maskp = np.ones((BL, SP), dtype=np.uint8)  # 1 = padded (masked out)
        for b in range(BL):
            gb = BL * ci + b
            idx = np.nonzero(masks[:, gb])[0]
            n = idx.shape[0]
            sel = hs[idx, gb, :].astype(np.float16)  # (n, H)
            hst_p[b, :, :n] = sel.T
            hsn_p[:n, b, :] = sel
            maskp[b, :n] = 0
        in_maps.append({
            "hst": np.ascontiguousarray(hst_p),
            "hsn": np.ascontiguousarray(hsn_p),
            "w1r": w1r,
            "w2t": w2t,
            "bvt": np.ascontiguousarray(bvt_base),
            "hidr": hidr,
            "masku": maskp.reshape(1, BL * SP),
        })
    return in_maps


def kernel(**inputs):
    SP = pick_sp(inputs["input_masks"])
    nc = build_program(SP)
    in_maps = prep_in_maps(inputs, SP)
    res = bass_utils.run_bass_kernel_spmd(nc, in_maps, list(range(NCORES)))
    out = np.concatenate([res.results[i]["ctx"] for i in range(NCORES)], axis=0)
    return out[None].astype(np.float32)


if __name__ == "__main__":
    build_program()
    print("program built OK")
